# revision 5
# baseline (speedup 1.0000x reference)
"""Cross-attention LLM block on 8 Trainium2 NeuronCores.

Sharding: core c handles batch b = c//2 and query-row half h = c%2
(2048 of the 4096 query rows of that batch), for ALL 16 heads.
K/V projections for a batch are computed redundantly by the two cores
sharing that batch; no cross-core communication.

All device matmuls are bf16 at 1 cycle/row. The host pre-transposes
and pre-packs activations and weights so the device does zero PE
transposes, and all DMA reads are contiguous per partition.

Per-core dataflow:
  phase 1 (persistent K^T and V in SBUF):
    kT[dh, h, t] = wk_h.T-chunks @ xkvT   (+bk via DVE)
    v [t, d]     = xkvT.T-chunks @ wv_g   (+bv via DVE, bf16)
  phase 2, per s-block of 512 query rows, per head (sw-pipelined):
    qT[dh, s]  = wq_h.T-chunks @ xqT      (+bq via DVE)
    scT[t, s]  = kT_h.T @ qT              (8 matmuls per 128-t-chunk)
    e_t        = exp(scT * 1/sqrt(128))   (ACT, bf16)
    Esum       = sum_t e_t                (DVE tree, bf16)
    den[1, s]  = ones.T @ Esum            (1 matmul)
    ctxT[dh,s] = v_h.T @ e_t              (accumulated over t-chunks)
    ctx_h      = ctxT * broadcast(1/den)  (DVE)
  out[s128, dg512] = sum_h ctx_h.T @ wo_hg  (+bo via DVE)
The denominator matmul of head h-1 and the ctx matmuls of head h-1
are issued between head h's Q-projection and head h's score matmuls,
and the out-projection of s-block sb-1 is issued inside head 0 of
s-block sb, so the Tensor engine never idles (keeps the HAM clock
gate at 2.4 GHz).
"""

import math
import sys

for _p in ("/opt/trn_rl_repo",):
    if _p not in sys.path:
        sys.path.append(_p)

import numpy as np

import concourse.bass as bass
import concourse.mybir as mybir
import concourse.tile as tile
from concourse import bacc
from concourse.bass_utils import run_bass_kernel_spmd

F32 = mybir.dt.float32
BF16 = mybir.dt.bfloat16

# full-problem dims
B, S_FULL, T_FULL, D_MODEL, NUM_HEADS = 4, 4096, 1024, 2048, 16
HEAD_DIM = 128
N_CORES = 8
S_LOC = (B * S_FULL) // N_CORES  # 2048 query rows per core


def build_program(S=S_LOC, T=T_FULL, D=D_MODEL, H=NUM_HEADS):
    """Build + compile the single-core program (SPMD across 8 cores)."""
    DH = HEAD_DIM
    NIC = D // 128          # contraction chunks (16)
    TH = T // 2             # t-half for K psum
    NTC = T // 128          # t-chunks (8)
    SB = min(512, S)        # s-block
    NSB = S // SB           # 4
    NJ = SB // 128          # 4
    NVG = D // 512          # 4
    NOG = D // 512          # 4
    ISCALE = 1.0 / math.sqrt(DH)
    ADD = mybir.AluOpType.add
    MULT = mybir.AluOpType.mult

    nc = bacc.Bacc("TRN2", target_bir_lowering=False, debug=False,
                   num_devices=N_CORES)

    # host-packed inputs (see make_in_maps for layouts)
    xq = nc.dram_tensor("xq", [NSB, 128, NIC, SB], BF16, kind="ExternalInput")
    xkv = nc.dram_tensor("xkv", [128, NIC, T], BF16, kind="ExternalInput")
    wq = nc.dram_tensor("wq", [H, 128, NIC, DH], BF16, kind="ExternalInput")
    wk = nc.dram_tensor("wk", [H, 128, NIC, DH], BF16, kind="ExternalInput")
    wv = nc.dram_tensor("wv", [NVG, 128, NIC, 512], BF16, kind="ExternalInput")
    wo = nc.dram_tensor("wo", [NOG, H, DH, 512], BF16, kind="ExternalInput")
    bqd = nc.dram_tensor("bq", [D], F32, kind="ExternalInput")
    bkd = nc.dram_tensor("bk", [D], F32, kind="ExternalInput")
    bvd = nc.dram_tensor("bv", [D], F32, kind="ExternalInput")
    bod = nc.dram_tensor("bo", [D], F32, kind="ExternalInput")
    out = nc.dram_tensor("out", [S, D], F32, kind="ExternalOutput")
    out_v = out.ap().rearrange("(n p) (g dg) -> n p g dg", p=128, dg=512)

    from contextlib import ExitStack
    with tile.TileContext(nc) as tc, ExitStack() as es:
        const = es.enter_context(tc.tile_pool(name="const", bufs=1))
        persist = es.enter_context(tc.tile_pool(name="persist", bufs=1))
        psum = es.enter_context(tc.tile_pool(name="psum", bufs=1, space="PSUM"))

        ones_bf = const.tile([128, 1], BF16)
        nc.gpsimd.memset(ones_bf[:], 1.0)
        bq_col = const.tile([128, H], F32)
        bk_col = const.tile([128, H], F32)
        nc.sync.dma_start(bq_col[:], bqd.ap().rearrange("(h p) -> p h", p=128))
        nc.sync.dma_start(bk_col[:], bkd.ap().rearrange("(h p) -> p h", p=128))
        # persistent K^T and V in SBUF
        kT = persist.tile([128, H, T], BF16)      # [dh, h, t]
        v_sb = persist.tile([128, NTC, D], BF16)  # [t%128, tc, d]

        # ---------------- phase 1: K^T and V ----------------
        with tc.tile_pool(name="ph1", bufs=1) as ph1:
            bv_row = ph1.tile([1, D], F32)
            nc.sync.dma_start(bv_row[:], bvd.ap()[None, :])
            bv_bc = ph1.tile([128, D], F32)
            nc.gpsimd.partition_broadcast(bv_bc[:], bv_row[:], channels=128)
            xkvT = ph1.tile([128, NIC, T], BF16)
            for c in range(NIC):
                nc.sync.dma_start(xkvT[:, c, :], xkv.ap()[:, c, :])

            wk_t = {}

            def dma_wk(h):
                wk_t[h] = ph1.tile([128, NIC, DH], BF16, tag="wk", bufs=2, name=f"wk{h}")
                nc.sync.dma_start(wk_t[h][:], wk.ap()[h])

            dma_wk(0)
            # prefetch phase-2 streams during phase 1
            xq_t = {}

            def dma_xq(sb):
                xq_t[sb] = persist.tile([128, NIC, SB], BF16, tag="xqs",
                                        bufs=2, name=f"xqs{sb}")
                nc.sync.dma_start(xq_t[sb][:], xq.ap()[sb])

            wq_t = {}

            def dma_wq(key, h):
                wq_t[key] = persist.tile([128, NIC, DH], BF16, tag="wq",
                                         bufs=2, name=f"wq{h}")
                nc.sync.dma_start(wq_t[key][:], wq.ap()[h])

            for h in range(H):
                if h + 1 < H:
                    dma_wk(h + 1)
                for half in range(2):
                    pk = psum.tile([128, TH], F32, tag="M", bufs=2)
                    for c in range(NIC):
                        nc.tensor.matmul(pk[:], wk_t[h][:, c, :],
                                         xkvT[:, c, half * TH:(half + 1) * TH],
                                         start=(c == 0), stop=(c == NIC - 1))
                    nc.vector.tensor_scalar(
                        kT[:, h, half * TH:(half + 1) * TH], pk[:],
                        bk_col[:, h:h + 1], None, ADD)
                del wk_t[h]

            dma_xq(0)
            dma_wq((0, 0), 0)

            wv_t = {}

            def dma_wv(g):
                wv_t[g] = ph1.tile([128, NIC, 512], BF16, tag="wv", bufs=2, name=f"wv{g}")
                nc.sync.dma_start(wv_t[g][:], wv.ap()[g])

            dma_wv(0)
            for g in range(NVG):
                if g + 1 < NVG:
                    dma_wv(g + 1)
                for tj in range(NTC):
                    pv = psum.tile([128, 512], F32, tag="M", bufs=2)
                    for c in range(NIC):
                        nc.tensor.matmul(pv[:],
                                         xkvT[:, c, tj * 128:(tj + 1) * 128],
                                         wv_t[g][:, c, :],
                                         start=(c == 0), stop=(c == NIC - 1))
                    nc.vector.tensor_tensor(
                        v_sb[:, tj, g * 512:(g + 1) * 512], pv[:],
                        bv_bc[:, g * 512:(g + 1) * 512], ADD)
                del wv_t[g]

        # ---------------- phase 2: attention + out projection --------
        with tc.tile_pool(name="ph2", bufs=1) as ph2:
            bo_row = ph2.tile([1, D], F32)
            nc.sync.dma_start(bo_row[:], bod.ap()[None, :])
            bo_bc = ph2.tile([128, D], F32)
            nc.gpsimd.partition_broadcast(bo_bc[:], bo_row[:], channels=128)
            ctxs = {}   # (sb, h) -> ctx tile [dh, s]
            exps = {}   # (h, t) -> exp tile [t128, s]
            esums = {}  # h -> Esum tile

            def den_ctx(sb, h):
                """Denominator + ctx matmuls for head h (issued later,
                between the next head's Q-projection and scores)."""
                pden = psum.tile([1, SB], F32, tag="M", bufs=2)
                nc.tensor.matmul(pden[:], ones_bf[:], esums[h][:])
                recip = ph2.tile([1, SB], F32, tag="recip", bufs=2)
                nc.vector.reciprocal(recip[:], pden[:])
                rden = ph2.tile([128, SB], F32, tag="rden", bufs=2)
                nc.gpsimd.partition_broadcast(rden[:], recip[:], channels=128)
                pctx = psum.tile([128, SB], F32, tag="C", bufs=2)
                for t in range(NTC):
                    nc.tensor.matmul(pctx[:],
                                     v_sb[:, t, h * DH:(h + 1) * DH],
                                     exps[(h, t)][:],
                                     start=(t == 0), stop=(t == NTC - 1))
                ctx_h = ph2.tile([128, SB], BF16, tag=f"ctx{h}", bufs=2)
                nc.vector.tensor_tensor(ctx_h[:], pctx[:], rden[:], MULT)
                ctxs[(sb, h)] = ctx_h
                del esums[h]
                for t in range(NTC):
                    del exps[(h, t)]

            def outproj(sb):
                for g in range(NOG):
                    pos = []
                    for j in range(NJ):
                        po_j = psum.tile([128, 512], F32, tag="B", bufs=4)
                        pos.append(po_j)
                    for h in range(H):
                        wo_hg = ph2.tile([128, 512], BF16, tag="wo", bufs=4)
                        nc.sync.dma_start(wo_hg[:], wo.ap()[g, h])
                        ctx_h = ctxs[(sb, h)]
                        for j in range(NJ):
                            nc.tensor.matmul(
                                pos[j][:], ctx_h[:, j * 128:(j + 1) * 128],
                                wo_hg[:],
                                start=(h == 0), stop=(h == H - 1))
                    for j in range(NJ):
                        o_sb = ph2.tile([128, 512], F32, tag="osb", bufs=2)
                        nc.vector.tensor_tensor(
                            o_sb[:], pos[j][:],
                            bo_bc[:, g * 512:(g + 1) * 512], ADD)
                        nc.sync.dma_start(out_v[sb * NJ + j, :, g, :],
                                          o_sb[:])
                for h in range(H):
                    del ctxs[(sb, h)]

            prev = None
            for sb in range(NSB):
                if sb + 1 < NSB:
                    dma_xq(sb + 1)
                for h in range(H):
                    # prefetch next head's Q weights
                    if h + 1 < H:
                        dma_wq((sb, h + 1), h + 1)
                    elif sb + 1 < NSB:
                        dma_wq((sb + 1, 0), 0)
                    # Q projection for head h
                    pq = psum.tile([128, SB], F32, tag="M", bufs=2)
                    for c in range(NIC):
                        nc.tensor.matmul(pq[:], wq_t[(sb, h)][:, c, :],
                                         xq_t[sb][:, c, :],
                                         start=(c == 0), stop=(c == NIC - 1))
                    del wq_t[(sb, h)]
                    qT = ph2.tile([128, SB], BF16, tag="qT", bufs=2)
                    nc.vector.tensor_scalar(qT[:], pq[:], bq_col[:, h:h + 1],
                                            None, ADD)
                    # pipelined work from earlier iterations
                    if prev is not None:
                        den_ctx(*prev)
                        prev = None
                    if h == 0 and sb > 0:
                        outproj(sb - 1)
                    # scores + exp + running sum for head h
                    esum = ph2.tile([128, SB], BF16, tag="esum", bufs=2)
                    esums[h] = esum
                    for t in range(NTC):
                        psc = psum.tile([128, SB], F32, tag="B", bufs=4)
                        nc.tensor.matmul(psc[:],
                                         kT[:, h, t * 128:(t + 1) * 128],
                                         qT[:])
                        e_t = ph2.tile([128, SB], BF16, tag="exp", bufs=16)
                        nc.scalar.activation(e_t[:], psc[:],
                                             mybir.ActivationFunctionType.Exp,
                                             scale=ISCALE)
                        exps[(h, t)] = e_t
                        if t == 1:
                            nc.vector.tensor_tensor(esum[:], exps[(h, 0)][:],
                                                    e_t[:], ADD)
                        elif t > 1:
                            nc.vector.tensor_tensor(esum[:], esum[:],
                                                    e_t[:], ADD)
                    prev = (sb, h)
                del xq_t[sb]
            den_ctx(*prev)
            outproj(NSB - 1)

    nc.compile()
    return nc


_NC_CACHE = {}


def _get_program(S=S_LOC, T=T_FULL, D=D_MODEL, H=NUM_HEADS):
    key = (S, T, D, H)
    if key not in _NC_CACHE:
        _NC_CACHE[key] = build_program(S, T, D, H)
    return _NC_CACHE[key]


def make_in_maps(query, key_value, Wq, bq, Wk, bk, Wv, bv, Wo, bo):
    f = np.float32
    import ml_dtypes
    bf = ml_dtypes.bfloat16

    def c(a):
        return np.ascontiguousarray(a)

    # weight packs (shared across cores)
    wqt = np.asarray(Wq).T.astype(bf)  # [d_in, d_out]
    wkt = np.asarray(Wk).T.astype(bf)
    wvt = np.asarray(Wv).T.astype(bf)
    wot = np.asarray(Wo).T.astype(bf)
    shared = {
        # [h, p, c, dh] from [(c p), (h dh)]
        "wq": c(wqt.reshape(16, 128, 16, 128).transpose(2, 1, 0, 3)),
        "wk": c(wkt.reshape(16, 128, 16, 128).transpose(2, 1, 0, 3)),
        # [g, p, c, dg] from [(c p), (g dg)]
        "wv": c(wvt.reshape(16, 128, 4, 512).transpose(2, 1, 0, 3)),
        # [g, h, dh, dg] from [(h dh), (g dg)]
        "wo": c(wot.reshape(16, 128, 4, 512).transpose(2, 0, 1, 3)),
        "bq": np.asarray(bq, f), "bk": np.asarray(bk, f),
        "bv": np.asarray(bv, f), "bo": np.asarray(bo, f),
    }
    n_batch = query.shape[0]
    halves = N_CORES // n_batch
    s_loc = query.shape[1] // halves
    nsb = s_loc // 512
    in_maps = []
    kv_packs = {}
    for core in range(N_CORES):
        b, hf = core // halves, core % halves
        if b not in kv_packs:
            # [p, c, t] from [t, (c p)]
            xkvT = np.asarray(key_value[b]).astype(bf).T
            kv_packs[b] = c(xkvT.reshape(16, 128, -1).transpose(1, 0, 2))
        # [sb, p, c, s] from [s, (c p)]
        xqT = np.asarray(
            query[b, hf * s_loc:(hf + 1) * s_loc]).astype(bf).T
        xq_pack = c(xqT.reshape(16, 128, nsb, 512).transpose(2, 1, 0, 3))
        in_maps.append({"xq": xq_pack, "xkv": kv_packs[b], **shared})
    return in_maps


def run(inputs, trace=False, tmpdir=None):
    """Run the SPMD kernel; returns (full_output, BassKernelResults)."""
    query = np.asarray(inputs["query"])
    key_value = np.asarray(inputs["key_value"])
    nb, s_full, d = query.shape
    nc = _get_program(S=(nb * s_full) // N_CORES, T=key_value.shape[1], D=d,
                      H=d // HEAD_DIM)
    in_maps = make_in_maps(**inputs)
    res = run_bass_kernel_spmd(nc, in_maps, core_ids=list(range(N_CORES)),
                               trace=trace, tmpdir=tmpdir)
    halves = N_CORES // nb
    s_loc = s_full // halves
    out = np.empty((nb, s_full, d), np.float32)
    for c in range(N_CORES):
        b, hf = c // halves, c % halves
        out[b, hf * s_loc:(hf + 1) * s_loc] = res.results[c]["out"]
    return out, res


def kernel(**inputs) -> np.ndarray:
    out, _ = run(inputs, trace=False)
    return out


# revision 10
# speedup vs baseline: 1.2838x; 1.2838x over previous
"""Cross-attention LLM block on 8 Trainium2 NeuronCores.

Sharding: core c handles batch b = c//2 and query-row half h = c%2
(2048 of the 4096 query rows of that batch), for ALL 16 heads.
K/V projections for a batch are computed redundantly by the two cores
sharing that batch; no cross-core communication.

All device matmuls are bf16 at 1 cycle/row. The host pre-transposes
and pre-packs activations and weights so the device does zero PE
transposes, and all DMA reads are contiguous per partition.

Per-core dataflow:
  phase 1 (persistent K^T and V in SBUF):
    kT[dh, h, t] = wk_h.T-chunks @ xkvT   (+bk via DVE)
    v [t, d]     = xkvT.T-chunks @ wv_g   (+bv via DVE, bf16)
  phase 2, per s-block of 512 query rows, per head (sw-pipelined):
    qT[dh, s]  = wq_h.T-chunks @ xqT      (+bq via DVE)
    scT[t, s]  = kT_h.T @ qT              (8 matmuls per 128-t-chunk)
    e_t        = exp(scT * 1/sqrt(128))   (ACT, bf16)
    Esum       = sum_t e_t                (DVE tree, bf16)
    den[1, s]  = ones.T @ Esum            (1 matmul)
    ctxT[dh,s] = v_h.T @ e_t              (accumulated over t-chunks)
    ctx_h      = ctxT * broadcast(1/den)  (DVE)
  out[s128, dg512] = sum_h ctx_h.T @ wo_hg  (+bo via DVE)
The denominator matmul of head h-1 and the ctx matmuls of head h-1
are issued between head h's Q-projection and head h's score matmuls,
and the out-projection of s-block sb-1 is issued inside head 0 of
s-block sb, so the Tensor engine never idles (keeps the HAM clock
gate at 2.4 GHz).
"""

import math
import sys

for _p in ("/opt/trn_rl_repo",):
    if _p not in sys.path:
        sys.path.append(_p)

import numpy as np

import concourse.bass as bass
import concourse.mybir as mybir
import concourse.tile as tile
from concourse import bacc
from concourse.bass_utils import run_bass_kernel_spmd

F32 = mybir.dt.float32
BF16 = mybir.dt.bfloat16

# full-problem dims
B, S_FULL, T_FULL, D_MODEL, NUM_HEADS = 4, 4096, 1024, 2048, 16
HEAD_DIM = 128
N_CORES = 8
S_LOC = (B * S_FULL) // N_CORES  # 2048 query rows per core


def build_program(S=S_LOC, T=T_FULL, D=D_MODEL, H=NUM_HEADS):
    """Build + compile the single-core program (SPMD across 8 cores)."""
    DH = HEAD_DIM
    NIC = D // 128          # contraction chunks (16)
    TH = T // 2             # t-half for K psum
    NTC = T // 128          # t-chunks (8)
    SB = min(512, S)        # s-block
    NSB = S // SB           # 4
    NJ = SB // 128          # 4
    NVG = D // 512          # 4
    NOG = D // 512          # 4
    ISCALE = 1.0 / math.sqrt(DH)
    ADD = mybir.AluOpType.add
    MULT = mybir.AluOpType.mult

    nc = bacc.Bacc("TRN2", target_bir_lowering=False, debug=False,
                   num_devices=N_CORES)

    # host-packed inputs (see make_in_maps for layouts)
    xq = nc.dram_tensor("xq", [NSB, 128, NIC, SB], BF16, kind="ExternalInput")
    xkv = nc.dram_tensor("xkv", [128, NIC, T], BF16, kind="ExternalInput")
    wq = nc.dram_tensor("wq", [H, 128, NIC, DH], BF16, kind="ExternalInput")
    wk = nc.dram_tensor("wk", [H, 128, NIC, DH], BF16, kind="ExternalInput")
    wv = nc.dram_tensor("wv", [NVG, 128, NIC, 512], BF16, kind="ExternalInput")
    wo = nc.dram_tensor("wo", [NOG, H, DH, 512], BF16, kind="ExternalInput")
    bqd = nc.dram_tensor("bq", [D], F32, kind="ExternalInput")
    bkd = nc.dram_tensor("bk", [D], F32, kind="ExternalInput")
    bvd = nc.dram_tensor("bv", [D], F32, kind="ExternalInput")
    bod = nc.dram_tensor("bo", [D], F32, kind="ExternalInput")
    out = nc.dram_tensor("out", [S, D], F32, kind="ExternalOutput")
    out_v = out.ap().rearrange("(n p) (g dg) -> n p g dg", p=128, dg=512)

    from contextlib import ExitStack
    with tile.TileContext(nc) as tc, ExitStack() as es:
        const = es.enter_context(tc.tile_pool(name="const", bufs=1))
        persist = es.enter_context(tc.tile_pool(name="persist", bufs=1))
        psum = es.enter_context(tc.tile_pool(name="psum", bufs=1, space="PSUM"))

        ones_bf = const.tile([128, 1], BF16)
        nc.gpsimd.memset(ones_bf[:], 1.0)
        bq_col = const.tile([128, H], F32)
        bk_col = const.tile([128, H], F32)
        nc.sync.dma_start(bq_col[:], bqd.ap().rearrange("(h p) -> p h", p=128))
        nc.sync.dma_start(bk_col[:], bkd.ap().rearrange("(h p) -> p h", p=128))
        # pre-scaled q bias so scores psum needs no extra scale pass
        bqs_col = const.tile([128, H], F32)
        nc.vector.tensor_scalar(bqs_col[:], bq_col[:], ISCALE, None, MULT)
        # persistent K^T and V in SBUF
        kT = persist.tile([128, H, T], BF16)      # [dh, h, t]
        v_sb = persist.tile([128, NTC, D], BF16)  # [t%128, tc, d]

        # ---------------- phase 1: K^T and V ----------------
        with tc.tile_pool(name="ph1", bufs=1) as ph1:
            bv_row = ph1.tile([1, D], F32)
            nc.sync.dma_start(bv_row[:], bvd.ap()[None, :])
            bv_bc = ph1.tile([128, D], F32)
            for g in range(NVG):
                nc.gpsimd.partition_broadcast(
                    bv_bc[:, g * 512:(g + 1) * 512],
                    bv_row[:, g * 512:(g + 1) * 512], channels=128)
            xkvT = ph1.tile([128, NIC, T], BF16)
            for c in range(NIC):
                nc.sync.dma_start(xkvT[:, c, :], xkv.ap()[:, c, :])

            wk_t = {}

            def dma_wk(h):
                wk_t[h] = ph1.tile([128, NIC, DH], BF16, tag="wk", bufs=2, name=f"wk{h}")
                nc.sync.dma_start(wk_t[h][:], wk.ap()[h])

            dma_wk(0)
            # prefetch phase-2 streams during phase 1
            xq_t = {}

            def dma_xq(sb):
                xq_t[sb] = persist.tile([128, NIC, SB], BF16, tag="xqs",
                                        bufs=2, name=f"xqs{sb}")
                nc.sync.dma_start(xq_t[sb][:], xq.ap()[sb])

            wq_t = {}

            def dma_wq(key, h):
                wq_t[key] = persist.tile([128, NIC, DH], BF16, tag="wq",
                                         bufs=2, name=f"wq{h}")
                nc.sync.dma_start(wq_t[key][:], wq.ap()[h])

            for h in range(H):
                if h + 1 < H:
                    dma_wk(h + 1)
                for half in range(2):
                    pk = psum.tile([128, TH], F32, tag="M", bufs=2)
                    for c in range(NIC):
                        nc.tensor.matmul(pk[:], wk_t[h][:, c, :],
                                         xkvT[:, c, half * TH:(half + 1) * TH],
                                         start=(c == 0), stop=(c == NIC - 1))
                    nc.vector.tensor_scalar(
                        kT[:, h, half * TH:(half + 1) * TH], pk[:],
                        bk_col[:, h:h + 1], None, ADD)
                del wk_t[h]

            dma_xq(0)
            dma_wq((0, 0), 0)

            wv_t = {}

            def dma_wv(g):
                wv_t[g] = ph1.tile([128, NIC, 512], BF16, tag="wv", bufs=2, name=f"wv{g}")
                nc.sync.dma_start(wv_t[g][:], wv.ap()[g])

            dma_wv(0)
            for g in range(NVG):
                if g + 1 < NVG:
                    dma_wv(g + 1)
                for tj in range(NTC):
                    pv = psum.tile([128, 512], F32, tag="M", bufs=2)
                    for c in range(NIC):
                        nc.tensor.matmul(pv[:],
                                         xkvT[:, c, tj * 128:(tj + 1) * 128],
                                         wv_t[g][:, c, :],
                                         start=(c == 0), stop=(c == NIC - 1))
                    nc.vector.tensor_tensor(
                        v_sb[:, tj, g * 512:(g + 1) * 512], pv[:],
                        bv_bc[:, g * 512:(g + 1) * 512], ADD)
                del wv_t[g]

        # ---------------- phase 2: attention + out projection --------
        with tc.tile_pool(name="ph2", bufs=1) as ph2:
            bo_row = ph2.tile([1, D], F32)
            nc.sync.dma_start(bo_row[:], bod.ap()[None, :])
            bo_bc = ph2.tile([128, D], F32)
            for g in range(NOG):
                nc.gpsimd.partition_broadcast(
                    bo_bc[:, g * 512:(g + 1) * 512],
                    bo_row[:, g * 512:(g + 1) * 512], channels=128)
            ctxs = {}   # (sb, h) -> ctx tile [dh, s]
            exps = {}   # (h, t) -> exp tile [t128, s]
            esums = {}  # h -> Esum tile

            def den_ctx(sb, h):
                """Denominator + ctx matmuls for head h (issued later,
                between the next head's Q-projection and scores)."""
                pden = psum.tile([1, SB], F32, tag="M", bufs=2)
                nc.tensor.matmul(pden[:], ones_bf[:], esums[h][:])
                recip = ph2.tile([1, SB], F32, tag="recip", bufs=2)
                nc.vector.reciprocal(recip[:], pden[:])
                rden = ph2.tile([128, SB], F32, tag="rden", bufs=2)
                nc.gpsimd.partition_broadcast(rden[:], recip[:], channels=128)
                pctx = psum.tile([128, SB], F32, tag="C", bufs=2)
                for t in range(NTC):
                    nc.tensor.matmul(pctx[:],
                                     v_sb[:, t, h * DH:(h + 1) * DH],
                                     exps[(h, t)][:],
                                     start=(t == 0), stop=(t == NTC - 1))
                ctx_h = ph2.tile([128, SB], BF16, tag=f"ctx{h}", bufs=2)
                nc.vector.tensor_tensor(ctx_h[:], pctx[:], rden[:], MULT)
                ctxs[(sb, h)] = ctx_h
                del esums[h]
                for t in range(NTC):
                    del exps[(h, t)]

            def outproj(sb):
                for g in range(NOG):
                    pos = []
                    for j in range(NJ):
                        po_j = psum.tile([128, 512], F32, tag="B", bufs=4)
                        pos.append(po_j)
                    for h in range(H):
                        wo_hg = ph2.tile([128, 512], BF16, tag="wo", bufs=4)
                        nc.sync.dma_start(wo_hg[:], wo.ap()[g, h])
                        ctx_h = ctxs[(sb, h)]
                        for j in range(NJ):
                            nc.tensor.matmul(
                                pos[j][:], ctx_h[:, j * 128:(j + 1) * 128],
                                wo_hg[:],
                                start=(h == 0), stop=(h == H - 1))
                    for j in range(NJ):
                        o_sb = ph2.tile([128, 512], F32, tag="osb", bufs=2)
                        nc.vector.tensor_tensor(
                            o_sb[:], pos[j][:],
                            bo_bc[:, g * 512:(g + 1) * 512], ADD)
                        nc.sync.dma_start(out_v[sb * NJ + j, :, g, :],
                                          o_sb[:])
                for h in range(H):
                    del ctxs[(sb, h)]

            prev = None
            for sb in range(NSB):
                if sb + 1 < NSB:
                    dma_xq(sb + 1)
                for h in range(H):
                    # prefetch next head's Q weights
                    if h + 1 < H:
                        dma_wq((sb, h + 1), h + 1)
                    elif sb + 1 < NSB:
                        dma_wq((sb + 1, 0), 0)
                    # Q projection for head h
                    pq = psum.tile([128, SB], F32, tag="M", bufs=2)
                    for c in range(NIC):
                        nc.tensor.matmul(pq[:], wq_t[(sb, h)][:, c, :],
                                         xq_t[sb][:, c, :],
                                         start=(c == 0), stop=(c == NIC - 1))
                    del wq_t[(sb, h)]
                    qT = ph2.tile([128, SB], BF16, tag="qT", bufs=2)
                    nc.vector.tensor_scalar(qT[:], pq[:], ISCALE,
                                            bqs_col[:, h:h + 1], MULT, ADD)
                    # pipelined work from earlier iterations
                    if prev is not None:
                        den_ctx(*prev)
                        prev = None
                    if h == 0 and sb > 0:
                        outproj(sb - 1)
                    # scores + exp + running sum for head h
                    esum = ph2.tile([128, SB], BF16, tag="esum", bufs=2)
                    esums[h] = esum
                    for t in range(NTC):
                        psc = psum.tile([128, SB], F32, tag="B", bufs=4)
                        nc.tensor.matmul(psc[:],
                                         kT[:, h, t * 128:(t + 1) * 128],
                                         qT[:])
                        e_t = ph2.tile([128, SB], BF16, tag="exp", bufs=16)
                        nc.scalar.activation(e_t[:], psc[:],
                                             mybir.ActivationFunctionType.Exp)
                        exps[(h, t)] = e_t
                        if t == 1:
                            nc.vector.tensor_tensor(esum[:], exps[(h, 0)][:],
                                                    e_t[:], ADD)
                        elif t > 1:
                            nc.vector.tensor_tensor(esum[:], esum[:],
                                                    e_t[:], ADD)
                    prev = (sb, h)
                del xq_t[sb]
            den_ctx(*prev)
            outproj(NSB - 1)

    nc.compile()
    return nc


_NC_CACHE = {}


def _get_program(S=S_LOC, T=T_FULL, D=D_MODEL, H=NUM_HEADS):
    key = (S, T, D, H)
    if key not in _NC_CACHE:
        _NC_CACHE[key] = build_program(S, T, D, H)
    return _NC_CACHE[key]


def make_in_maps(query, key_value, Wq, bq, Wk, bk, Wv, bv, Wo, bo):
    f = np.float32
    import ml_dtypes
    bf = ml_dtypes.bfloat16

    def c(a):
        return np.ascontiguousarray(a)

    # weight packs (shared across cores)
    wqt = np.asarray(Wq).T.astype(bf)  # [d_in, d_out]
    wkt = np.asarray(Wk).T.astype(bf)
    wvt = np.asarray(Wv).T.astype(bf)
    wot = np.asarray(Wo).T.astype(bf)
    shared = {
        # [h, p, c, dh] from [(c p), (h dh)]
        "wq": c(wqt.reshape(16, 128, 16, 128).transpose(2, 1, 0, 3)),
        "wk": c(wkt.reshape(16, 128, 16, 128).transpose(2, 1, 0, 3)),
        # [g, p, c, dg] from [(c p), (g dg)]
        "wv": c(wvt.reshape(16, 128, 4, 512).transpose(2, 1, 0, 3)),
        # [g, h, dh, dg] from [(h dh), (g dg)]
        "wo": c(wot.reshape(16, 128, 4, 512).transpose(2, 0, 1, 3)),
        "bq": np.asarray(bq, f), "bk": np.asarray(bk, f),
        "bv": np.asarray(bv, f), "bo": np.asarray(bo, f),
    }
    n_batch = query.shape[0]
    halves = N_CORES // n_batch
    s_loc = query.shape[1] // halves
    nsb = s_loc // 512
    in_maps = []
    kv_packs = {}
    for core in range(N_CORES):
        b, hf = core // halves, core % halves
        if b not in kv_packs:
            # [p, c, t] from [t, (c p)]
            xkvT = np.asarray(key_value[b]).astype(bf).T
            kv_packs[b] = c(xkvT.reshape(16, 128, -1).transpose(1, 0, 2))
        # [sb, p, c, s] from [s, (c p)]
        xqT = np.asarray(
            query[b, hf * s_loc:(hf + 1) * s_loc]).astype(bf).T
        xq_pack = c(xqT.reshape(16, 128, nsb, 512).transpose(2, 1, 0, 3))
        in_maps.append({"xq": xq_pack, "xkv": kv_packs[b], **shared})
    return in_maps


def run(inputs, trace=False, tmpdir=None):
    """Run the SPMD kernel; returns (full_output, BassKernelResults)."""
    query = np.asarray(inputs["query"])
    key_value = np.asarray(inputs["key_value"])
    nb, s_full, d = query.shape
    nc = _get_program(S=(nb * s_full) // N_CORES, T=key_value.shape[1], D=d,
                      H=d // HEAD_DIM)
    in_maps = make_in_maps(**inputs)
    res = run_bass_kernel_spmd(nc, in_maps, core_ids=list(range(N_CORES)),
                               trace=trace, tmpdir=tmpdir)
    halves = N_CORES // nb
    s_loc = s_full // halves
    out = np.empty((nb, s_full, d), np.float32)
    for c in range(N_CORES):
        b, hf = c // halves, c % halves
        out[b, hf * s_loc:(hf + 1) * s_loc] = res.results[c]["out"]
    return out, res


def kernel(**inputs) -> np.ndarray:
    out, _ = run(inputs, trace=False)
    return out


# revision 13
# speedup vs baseline: 1.4991x; 1.1677x over previous
"""Cross-attention LLM block on 8 Trainium2 NeuronCores.

Sharding: core c handles batch b = c//2 and query-row half h = c%2
(2048 of the 4096 query rows of that batch), for ALL 16 heads.
K/V projections for a batch are computed redundantly by the two cores
sharing that batch; no cross-core communication.

All device matmuls are bf16 at 1 cycle/row. The host pre-transposes
and pre-packs activations and weights so the device does zero PE
transposes, and all DMA reads are contiguous per partition.

Per-core dataflow:
  phase 1 (persistent K^T and V in SBUF):
    kT[dh, h, t] = wk_h.T-chunks @ xkvT   (+bk via DVE)
    v [t, d]     = xkvT.T-chunks @ wv_g   (+bv via DVE, bf16)
  phase 2, per s-block of 512 query rows, per head (sw-pipelined):
    qT[dh, s]  = wq_h.T-chunks @ xqT      (+bq via DVE)
    scT[t, s]  = kT_h.T @ qT              (8 matmuls per 128-t-chunk)
    e_t        = exp(scT * 1/sqrt(128))   (ACT, bf16)
    Esum       = sum_t e_t                (DVE tree, bf16)
    den[1, s]  = ones.T @ Esum            (1 matmul)
    ctxT[dh,s] = v_h.T @ e_t              (accumulated over t-chunks)
    ctx_h      = ctxT * broadcast(1/den)  (DVE)
  out[s128, dg512] = sum_h ctx_h.T @ wo_hg  (+bo via DVE)
The denominator matmul of head h-1 and the ctx matmuls of head h-1
are issued between head h's Q-projection and head h's score matmuls,
and the out-projection of s-block sb-1 is issued inside head 0 of
s-block sb, so the Tensor engine never idles (keeps the HAM clock
gate at 2.4 GHz).
"""

import math
import sys

for _p in ("/opt/trn_rl_repo",):
    if _p not in sys.path:
        sys.path.append(_p)

import numpy as np

import concourse.bass as bass
import concourse.mybir as mybir
import concourse.tile as tile
from concourse import bacc
from concourse.bass_utils import run_bass_kernel_spmd

F32 = mybir.dt.float32
BF16 = mybir.dt.bfloat16

# full-problem dims
B, S_FULL, T_FULL, D_MODEL, NUM_HEADS = 4, 4096, 1024, 2048, 16
HEAD_DIM = 128
N_CORES = 8
S_LOC = (B * S_FULL) // N_CORES  # 2048 query rows per core


def build_program(S=S_LOC, T=T_FULL, D=D_MODEL, H=NUM_HEADS):
    """Build + compile the single-core program (SPMD across 8 cores)."""
    DH = HEAD_DIM
    NIC = D // 128          # contraction chunks (16)
    TH = T // 2             # t-half for K psum
    NTC = T // 128          # t-chunks (8)
    SB = min(512, S)        # s-block
    NSB = S // SB           # 4
    NJ = SB // 128          # 4
    NVG = D // 512          # 4
    NOG = D // 512          # 4
    ISCALE = 1.0 / math.sqrt(DH)
    ADD = mybir.AluOpType.add
    MULT = mybir.AluOpType.mult

    nc = bacc.Bacc("TRN2", target_bir_lowering=False, debug=False,
                   num_devices=N_CORES)

    # host-packed inputs (see make_in_maps for layouts)
    xq = nc.dram_tensor("xq", [NSB, 128, NIC, SB], BF16, kind="ExternalInput")
    xkv = nc.dram_tensor("xkv", [128, NIC, T], BF16, kind="ExternalInput")
    wq = nc.dram_tensor("wq", [H, 128, NIC, DH], BF16, kind="ExternalInput")
    wk = nc.dram_tensor("wk", [H, 128, NIC, DH], BF16, kind="ExternalInput")
    wv = nc.dram_tensor("wv", [NVG, 128, NIC, 512], BF16, kind="ExternalInput")
    wo = nc.dram_tensor("wo", [NOG, H, DH, 512], BF16, kind="ExternalInput")
    bqd = nc.dram_tensor("bq", [D], F32, kind="ExternalInput")
    bkd = nc.dram_tensor("bk", [D], F32, kind="ExternalInput")
    bvd = nc.dram_tensor("bv", [D], F32, kind="ExternalInput")
    bod = nc.dram_tensor("bo", [D], F32, kind="ExternalInput")
    out = nc.dram_tensor("out", [S, D], F32, kind="ExternalOutput")
    out_v = out.ap().rearrange("(n p) (g dg) -> n p g dg", p=128, dg=512)

    from contextlib import ExitStack
    with tile.TileContext(nc) as tc, ExitStack() as es:
        const = es.enter_context(tc.tile_pool(name="const", bufs=1))
        persist = es.enter_context(tc.tile_pool(name="persist", bufs=1))
        psum = es.enter_context(tc.tile_pool(name="psum", bufs=1, space="PSUM"))

        ones_bf = const.tile([128, 1], BF16)
        nc.gpsimd.memset(ones_bf[:], 1.0)
        bq_col = const.tile([128, H], F32)
        bk_col = const.tile([128, H], F32)
        nc.sync.dma_start(bq_col[:], bqd.ap().rearrange("(h p) -> p h", p=128))
        nc.sync.dma_start(bk_col[:], bkd.ap().rearrange("(h p) -> p h", p=128))
        # pre-scaled q bias so scores psum needs no extra scale pass
        bqs_col = const.tile([128, H], F32)
        nc.vector.tensor_scalar(bqs_col[:], bq_col[:], ISCALE, None, MULT)
        # persistent K^T and V in SBUF
        kT = persist.tile([128, H, T], BF16)      # [dh, h, t]
        v_sb = persist.tile([128, NTC, D], BF16)  # [t%128, tc, d]

        # ---------------- phase 1: K^T and V ----------------
        with tc.tile_pool(name="ph1", bufs=1) as ph1:
            bv_row = ph1.tile([1, D], F32)
            nc.sync.dma_start(bv_row[:], bvd.ap()[None, :])
            bv_bc = ph1.tile([128, D], F32)
            for g in range(NVG):
                nc.gpsimd.partition_broadcast(
                    bv_bc[:, g * 512:(g + 1) * 512],
                    bv_row[:, g * 512:(g + 1) * 512], channels=128)
            xkvT = ph1.tile([128, NIC, T], BF16)
            for c in range(NIC):
                nc.sync.dma_start(xkvT[:, c, :], xkv.ap()[:, c, :])

            wk_t = {}

            def dma_wk(h):
                wk_t[h] = ph1.tile([128, NIC, DH], BF16, tag="wk", bufs=2, name=f"wk{h}")
                nc.sync.dma_start(wk_t[h][:], wk.ap()[h])

            dma_wk(0)
            # prefetch phase-2 streams during phase 1
            xq_t = {}

            def dma_xq(sb):
                xq_t[sb] = persist.tile([128, NIC, SB], BF16, tag="xqs",
                                        bufs=2, name=f"xqs{sb}")
                nc.sync.dma_start(xq_t[sb][:], xq.ap()[sb])

            wq_t = {}

            def dma_wq(key, h):
                wq_t[key] = persist.tile([128, NIC, DH], BF16, tag="wq",
                                         bufs=2, name=f"wq{h}")
                nc.sync.dma_start(wq_t[key][:], wq.ap()[h])

            for h in range(H):
                if h + 1 < H:
                    dma_wk(h + 1)
                for half in range(2):
                    pk = psum.tile([128, TH], F32, tag="M", bufs=2)
                    for c in range(NIC):
                        nc.tensor.matmul(pk[:], wk_t[h][:, c, :],
                                         xkvT[:, c, half * TH:(half + 1) * TH],
                                         start=(c == 0), stop=(c == NIC - 1))
                    nc.vector.tensor_scalar(
                        kT[:, h, half * TH:(half + 1) * TH], pk[:],
                        bk_col[:, h:h + 1], None, ADD)
                del wk_t[h]

            dma_xq(0)
            dma_wq((0, 0), 0)

            wv_t = {}

            def dma_wv(g):
                wv_t[g] = ph1.tile([128, NIC, 512], BF16, tag="wv", bufs=2, name=f"wv{g}")
                nc.sync.dma_start(wv_t[g][:], wv.ap()[g])

            dma_wv(0)
            for g in range(NVG):
                if g + 1 < NVG:
                    dma_wv(g + 1)
                for tj in range(NTC):
                    pv = psum.tile([128, 512], F32, tag="M", bufs=2)
                    for c in range(NIC):
                        nc.tensor.matmul(pv[:],
                                         xkvT[:, c, tj * 128:(tj + 1) * 128],
                                         wv_t[g][:, c, :],
                                         start=(c == 0), stop=(c == NIC - 1))
                    nc.vector.tensor_tensor(
                        v_sb[:, tj, g * 512:(g + 1) * 512], pv[:],
                        bv_bc[:, g * 512:(g + 1) * 512], ADD)
                del wv_t[g]

        # ---------------- phase 2: attention + out projection --------
        with tc.tile_pool(name="ph2", bufs=1) as ph2:
            bo_row = ph2.tile([1, D], F32)
            nc.sync.dma_start(bo_row[:], bod.ap()[None, :])
            bo_bc = ph2.tile([128, D], F32)
            for g in range(NOG):
                nc.gpsimd.partition_broadcast(
                    bo_bc[:, g * 512:(g + 1) * 512],
                    bo_row[:, g * 512:(g + 1) * 512], channels=128)
            ctxs = {}   # (sb, h) -> ctx tile [dh, s]
            exps = {}   # (h, t) -> exp tile [t128, s]

            def den_ctx(sb, h):
                """Denominator + ctx matmuls for head h (issued later,
                between the next head's Q-projection and scores)."""
                pden = psum.tile([1, SB], F32, tag="M", bufs=2)
                pctx = psum.tile([128, SB], F32, tag="C", bufs=2)
                for t in range(NTC):
                    nc.tensor.matmul(pden[:], ones_bf[:], exps[(h, t)][:],
                                     start=(t == 0), stop=(t == NTC - 1))
                    nc.tensor.matmul(pctx[:],
                                     v_sb[:, t, h * DH:(h + 1) * DH],
                                     exps[(h, t)][:],
                                     start=(t == 0), stop=(t == NTC - 1))
                recip = ph2.tile([1, SB], F32, tag="recip", bufs=2)
                nc.vector.reciprocal(recip[:], pden[:])
                rden = ph2.tile([128, SB], F32, tag="rden", bufs=2)
                nc.gpsimd.partition_broadcast(rden[:], recip[:], channels=128)
                ctx_h = ph2.tile([128, SB], BF16, tag=f"ctx{h}", bufs=2)
                nc.vector.tensor_tensor(ctx_h[:], pctx[:], rden[:], MULT)
                ctxs[(sb, h)] = ctx_h
                for t in range(NTC):
                    del exps[(h, t)]

            def outproj(sb):
                for g in range(NOG):
                    pos = []
                    for j in range(NJ):
                        po_j = psum.tile([128, 512], F32, tag="B", bufs=4)
                        pos.append(po_j)
                    for h in range(H):
                        wo_hg = ph2.tile([128, 512], BF16, tag="wo", bufs=4)
                        nc.sync.dma_start(wo_hg[:], wo.ap()[g, h])
                        ctx_h = ctxs[(sb, h)]
                        for j in range(NJ):
                            nc.tensor.matmul(
                                pos[j][:], ctx_h[:, j * 128:(j + 1) * 128],
                                wo_hg[:],
                                start=(h == 0), stop=(h == H - 1))
                    for j in range(NJ):
                        o_sb = ph2.tile([128, 512], F32, tag="osb", bufs=2)
                        nc.vector.tensor_tensor(
                            o_sb[:], pos[j][:],
                            bo_bc[:, g * 512:(g + 1) * 512], ADD)
                        nc.sync.dma_start(out_v[sb * NJ + j, :, g, :],
                                          o_sb[:])
                for h in range(H):
                    del ctxs[(sb, h)]

            prev = None
            for sb in range(NSB):
                if sb + 1 < NSB:
                    dma_xq(sb + 1)
                for h in range(H):
                    # prefetch next head's Q weights
                    if h + 1 < H:
                        dma_wq((sb, h + 1), h + 1)
                    elif sb + 1 < NSB:
                        dma_wq((sb + 1, 0), 0)
                    # Q projection for head h
                    pq = psum.tile([128, SB], F32, tag="M", bufs=2)
                    for c in range(NIC):
                        nc.tensor.matmul(pq[:], wq_t[(sb, h)][:, c, :],
                                         xq_t[sb][:, c, :],
                                         start=(c == 0), stop=(c == NIC - 1))
                    del wq_t[(sb, h)]
                    qT = ph2.tile([128, SB], BF16, tag="qT", bufs=2)
                    nc.vector.tensor_scalar(qT[:], pq[:], ISCALE,
                                            bqs_col[:, h:h + 1], MULT, ADD)
                    # pipelined work from earlier iterations
                    if prev is not None:
                        den_ctx(*prev)
                        prev = None
                    if h == 0 and sb > 0:
                        outproj(sb - 1)
                    # scores + exp for head h
                    for t in range(NTC):
                        psc = psum.tile([128, SB], F32, tag="B", bufs=4)
                        nc.tensor.matmul(psc[:],
                                         kT[:, h, t * 128:(t + 1) * 128],
                                         qT[:])
                        e_t = ph2.tile([128, SB], BF16, tag="exp", bufs=16)
                        nc.scalar.activation(e_t[:], psc[:],
                                             mybir.ActivationFunctionType.Exp)
                        exps[(h, t)] = e_t
                    prev = (sb, h)
                del xq_t[sb]
            den_ctx(*prev)
            outproj(NSB - 1)

    nc.compile()
    return nc


_NC_CACHE = {}


def _get_program(S=S_LOC, T=T_FULL, D=D_MODEL, H=NUM_HEADS):
    key = (S, T, D, H)
    if key not in _NC_CACHE:
        _NC_CACHE[key] = build_program(S, T, D, H)
    return _NC_CACHE[key]


def make_in_maps(query, key_value, Wq, bq, Wk, bk, Wv, bv, Wo, bo):
    f = np.float32
    import ml_dtypes
    bf = ml_dtypes.bfloat16

    def c(a):
        return np.ascontiguousarray(a)

    # weight packs (shared across cores)
    wqt = np.asarray(Wq).T.astype(bf)  # [d_in, d_out]
    wkt = np.asarray(Wk).T.astype(bf)
    wvt = np.asarray(Wv).T.astype(bf)
    wot = np.asarray(Wo).T.astype(bf)
    shared = {
        # [h, p, c, dh] from [(c p), (h dh)]
        "wq": c(wqt.reshape(16, 128, 16, 128).transpose(2, 1, 0, 3)),
        "wk": c(wkt.reshape(16, 128, 16, 128).transpose(2, 1, 0, 3)),
        # [g, p, c, dg] from [(c p), (g dg)]
        "wv": c(wvt.reshape(16, 128, 4, 512).transpose(2, 1, 0, 3)),
        # [g, h, dh, dg] from [(h dh), (g dg)]
        "wo": c(wot.reshape(16, 128, 4, 512).transpose(2, 0, 1, 3)),
        "bq": np.asarray(bq, f), "bk": np.asarray(bk, f),
        "bv": np.asarray(bv, f), "bo": np.asarray(bo, f),
    }
    n_batch = query.shape[0]
    halves = N_CORES // n_batch
    s_loc = query.shape[1] // halves
    nsb = s_loc // 512
    in_maps = []
    kv_packs = {}
    for core in range(N_CORES):
        b, hf = core // halves, core % halves
        if b not in kv_packs:
            # [p, c, t] from [t, (c p)]
            xkvT = np.asarray(key_value[b]).astype(bf).T
            kv_packs[b] = c(xkvT.reshape(16, 128, -1).transpose(1, 0, 2))
        # [sb, p, c, s] from [s, (c p)]
        xqT = np.asarray(
            query[b, hf * s_loc:(hf + 1) * s_loc]).astype(bf).T
        xq_pack = c(xqT.reshape(16, 128, nsb, 512).transpose(2, 1, 0, 3))
        in_maps.append({"xq": xq_pack, "xkv": kv_packs[b], **shared})
    return in_maps


def run(inputs, trace=False, tmpdir=None):
    """Run the SPMD kernel; returns (full_output, BassKernelResults)."""
    query = np.asarray(inputs["query"])
    key_value = np.asarray(inputs["key_value"])
    nb, s_full, d = query.shape
    nc = _get_program(S=(nb * s_full) // N_CORES, T=key_value.shape[1], D=d,
                      H=d // HEAD_DIM)
    in_maps = make_in_maps(**inputs)
    res = run_bass_kernel_spmd(nc, in_maps, core_ids=list(range(N_CORES)),
                               trace=trace, tmpdir=tmpdir)
    halves = N_CORES // nb
    s_loc = s_full // halves
    out = np.empty((nb, s_full, d), np.float32)
    for c in range(N_CORES):
        b, hf = c // halves, c % halves
        out[b, hf * s_loc:(hf + 1) * s_loc] = res.results[c]["out"]
    return out, res


def kernel(**inputs) -> np.ndarray:
    out, _ = run(inputs, trace=False)
    return out
